# revision 10
# baseline (speedup 1.0000x reference)
"""GAT layer kernel for Trainium2 (8 NeuronCores, Bass/Tile).

Strategy:
  - Nodes are permuted by in-degree (host-side index preprocessing) so that
    128-node tiles have homogeneous degree; tiles are dealt round-robin to the
    8 cores so per-core edge counts balance and all cores share one SPMD
    instruction stream.
  - Phase A (on device): table[n] = [seq_fts(n) (128, fp16) | f1(n)+b1 |
    f2(n)+b2] in 512B rows (256 fp16 elements, cols 130..255 unwritten pad)
    built with one PE matmul chain per 128-node tile against the host-side
    augmented weight matrix W_aug = [W | W@a1 | W@a2]; bias adds fused into
    the psum->fp16 copy; table written to HBM 4 tiles per DMA.
  - Phase B (on device): the table is split into 4 windows of 25600 rows so
    gather indices fit int16; per node-tile round, one bulk dma_gather per
    window (mlp-library InstDMAGatherAnt, single_packet=False) fetches all of
    the round's window-w edge slots in one gpsimd instruction.  Slot columns
    are window-pure, padded to the round's global per-window max degree; pad
    slots point at a per-window dummy row with f2 = -60000 so exp() == 0.
    The dst tile's own window block is ordered last with a self slot as the
    final column, providing f1 without entering the softmax ranges.
    Softmax: ACT lrelu pieces -> ACT Exp with accum_out (denominator for
    free) -> one broadcast tensor_tensor multiply in place on the gathered
    features -> fold-halves fp16 add tree -> normalize-after-aggregation ->
    fused elu(elu(x)).
"""

import os
import numpy as np

# ---- problem constants (self-contained; must match reference.py) ----
N_NODES = 100000
N_EDGES = 1600000
IN_DIM = 256
OUT_DIM = 128
LRELU_ALPHA = 0.2

NCORES = 8
P = 128
ROW = OUT_DIM + 2      # written row prefix: 128 feats + f1 + f2
EL = 256               # table row stride in fp16 elements (512 bytes)
NW = 4                 # index windows
DUMMY_F2 = -60000.0

_last_results = {}


def _ceil_to(x, m):
    return (x + m - 1) // m * m


def _preprocess(dst, src, n, npad):
    """Host-side index preprocessing for the windowed dma_gather layout.

    Returns (order, rounds, Cmax, coff, TOTC, idx16_all) where rounds[r] is
    a list of (window, K) gather blocks in processing order (self window
    last), idx16_all[c] is core c's replicated int16 index stream.
    """
    WROW = npad // NW
    ntiles = npad // P
    R = ntiles // NCORES
    RPW = WROW // (P * NCORES)      # rounds per window (dst tiles)

    deg = np.bincount(dst, minlength=npad).astype(np.int64)
    order = np.argsort(-deg, kind="stable")          # permuted pos -> node

    # reserve the last position of each window for a dummy row: swap in the
    # four lowest-degree (pad) nodes
    dummy_pos = np.array([w * WROW + WROW - 1 for w in range(NW)])
    tail = np.array([npad - 1 - i for i in range(NW)])
    for dp, tl in zip(dummy_pos, tail):
        if dp != tl:
            order[[dp, tl]] = order[[tl, dp]]
    issrc = np.zeros(npad, dtype=bool)
    issrc[src] = True
    assert not issrc[order[dummy_pos]].any() and \
        not deg[order[dummy_pos]].any(), \
        "dummy-row positions must hold edge-free (pad) nodes"
    invpos = np.empty(npad, dtype=np.int64)
    invpos[order] = np.arange(npad)                  # node -> permuted pos

    pos_d = invpos[dst]
    pos_s = invpos[src]
    w_e = pos_s // WROW                              # window of each edge
    r_e = pos_d // (P * NCORES)                      # round of each edge

    # per (round, window) max count over the round's 1024 dsts
    dw = pos_d * NW + w_e
    cnt_dw = np.bincount(dw, minlength=npad * NW).reshape(npad, NW)
    K_rw = cnt_dw.reshape(R, P * NCORES, NW).max(axis=1)   # [R, NW]

    # per-round window order: self window last
    rounds = []
    C = np.zeros(R, dtype=np.int64)
    colbase = np.zeros((R, NW), dtype=np.int64)
    for r in range(R):
        wself = (r * P * NCORES) // WROW
        worder = [w for w in range(NW) if w != wself] + [wself]
        blocks = []
        cb = 0
        for w in worder:
            k = int(K_rw[r, w])
            if w == wself:
                k += 1                              # self slot column
            if k:
                blocks.append((w, int(K_rw[r, w]), cb))
            colbase[r, w] = cb
            cb += k
        C[r] = cb                                   # includes self column
        rounds.append(blocks)

    coff = np.zeros(R + 1, dtype=np.int64)
    np.cumsum(8 * C, out=coff[1:])
    TOTC = int(coff[-1])

    # slot index of each edge within (dst, window)
    ordE = np.argsort(dw, kind="stable")
    dw_s = dw[ordE]
    ps_s = pos_s[ordE]
    _, first, counts = np.unique(dw_s, return_index=True, return_counts=True)
    slot = np.arange(len(dw_s), dtype=np.int64) - np.repeat(first, counts)

    pd_s = dw_s // NW
    we_s = dw_s % NW
    r_s = pd_s // (P * NCORES)
    cc_s = (pd_s // P) % NCORES
    p_s = pd_s % P
    col = colbase[r_s, we_s] + slot
    # wrapped int16 position: i = col*128 + p -> [p%16, col*8 + p//16]
    row16 = p_s % 16
    col16 = coff[r_s] + col * 8 + p_s // 16
    val = (ps_s % WROW).astype(np.int16)

    idx16_all = np.full((NCORES, 16, TOTC), WROW - 1, dtype=np.int16)
    idx16_all[cc_s, row16, col16] = val

    # self slots: col = C[r]-1, val = own position % WROW
    rr = np.repeat(np.arange(R), P)
    pp = np.tile(np.arange(P), R)
    srow16 = pp % 16
    scol16 = coff[rr] + (C[rr] - 1) * 8 + pp // 16
    for cc in range(NCORES):
        own = (rr * NCORES + cc) * P + pp
        idx16_all[cc, srow16, scol16] = (own % WROW).astype(np.int16)

    idx16_rep = np.ascontiguousarray(
        np.tile(idx16_all, (1, 8, 1)))               # [NCORES, 128, TOTC]
    return order, rounds, C, coff, TOTC, idx16_rep


def _build_program(npad, rounds, C, coff, TOTC, in_dim, exp_shift):
    import concourse.bass as bass
    import concourse.tile as tile
    from concourse import bacc, mybir
    from concourse.library_config import mlp
    from contextlib import ExitStack

    f16 = mybir.dt.float16
    f32 = mybir.dt.float32
    i16 = mybir.dt.int16
    AF = mybir.ActivationFunctionType
    OP = mybir.AluOpType
    D = OUT_DIM
    KT = in_dim // P
    R = len(rounds)
    WROW = npad // NW
    Cmax = int(max(C))
    rows_per_core = R * P

    nc = bacc.Bacc("TRN2", target_bir_lowering=False, debug=False,
                   num_devices=NCORES)
    xt_h = nc.declare_dram_parameter("xt", [in_dim, npad], f16, isOutput=False)
    waug_h = nc.declare_dram_parameter("waug", [in_dim, ROW], f16,
                                       isOutput=False)
    brow_h = nc.declare_dram_parameter("brow", [1, ROW], f32, isOutput=False)
    bias_h = nc.declare_dram_parameter("bias1", [1, D], f32, isOutput=False)
    dummy_h = nc.declare_dram_parameter("dumrow", [1, ROW], f16, isOutput=False)
    sidx_h = nc.declare_dram_parameter("sidx", [P, TOTC], i16, isOutput=False)
    out_h = nc.declare_dram_parameter("out", [rows_per_core, D], f32,
                                      isOutput=True)

    table_h = nc.dram_tensor("table", [npad, EL], f16)

    with tile.TileContext(nc) as tc, ExitStack() as ctx:
        nc.gpsimd.load_library(mlp)
        cpool = ctx.enter_context(tc.tile_pool(name="consts", bufs=1))
        w_sb = [cpool.tile([P, ROW], f16, name=f"wsb{k}", tag=f"w{k}")
                for k in range(KT)]
        for k in range(KT):
            nc.sync.dma_start(out=w_sb[k][:], in_=waug_h[k * P:(k + 1) * P, :])
        brow_sb = cpool.tile([P, ROW], f32, tag="brow")
        nc.sync.dma_start(out=brow_sb[:],
                          in_=brow_h[0:1, :].to_broadcast([P, ROW]))
        bias_sb = cpool.tile([P, D], f32, tag="bias128")
        nc.sync.dma_start(out=bias_sb[:],
                          in_=bias_h[0:1, :].to_broadcast([P, D]))
        esh_sb = cpool.tile([P, 1], f32, tag="eshcol")
        nc.vector.memset(esh_sb[:], -float(exp_shift))
        zero_sb = cpool.tile([P, 1], f32, tag="zerocol")
        nc.vector.memset(zero_sb[:], 0.0)
        negone_sb = cpool.tile([P, 1], f32, tag="negonecol")
        nc.vector.memset(negone_sb[:], -1.0)
        dm_sb = cpool.tile([1, ROW], f16, tag="dummy")
        nc.sync.dma_start(out=dm_sb[:], in_=dummy_h[:, :])

        # fin0 = elu(elu(bias)) for empty rounds
        ob0 = cpool.tile([P, D], f16, tag="ob0")
        nc.vector.tensor_copy(out=ob0[:], in_=bias_sb[:])
        mm0 = cpool.tile([P, D], f16, tag="mm0")
        nc.vector.tensor_scalar(out=mm0[:], in0=ob0[:], scalar1=0.0,
                                scalar2=None, op0=OP.min)
        ex0 = cpool.tile([P, D], f16, tag="ex0")
        nc.scalar.activation(out=ex0[:], in_=mm0[:], func=AF.Exp,
                             bias=zero_sb[:, 0:1], scale=1.0)
        ex20 = cpool.tile([P, D], f16, tag="ex20")
        nc.scalar.activation(out=ex20[:], in_=ex0[:], func=AF.Exp,
                             bias=negone_sb[:, 0:1], scale=1.0)
        e10 = cpool.tile([P, D], f16, tag="e10")
        nc.vector.tensor_scalar(out=e10[:], in0=ex20[:], scalar1=-1.0,
                                scalar2=None, op0=OP.add)
        fin0 = cpool.tile([P, D], f32, tag="fin0")
        nc.vector.tensor_tensor(out=fin0[:], in0=ob0[:], in1=e10[:], op=OP.max)

        # ---------------- Phase A: build the table (node-major) ----------
        with nc.named_scope("phaseA"), ExitStack() as actx:
            xpool = actx.enter_context(tc.tile_pool(name="x", bufs=6))
            pspool = actx.enter_context(
                tc.tile_pool(name="psA", bufs=6, space="PSUM"))
            vpool = actx.enter_context(tc.tile_pool(name="vtile", bufs=4))

            ntile_all = npad // P
            for tb in range(ntile_all // 4):
                xks = []
                for k in range(KT):
                    xk = xpool.tile([P, 4 * P], f16, tag="xk", name=f"xk{k}")
                    nc.scalar.dma_start(
                        out=xk[:],
                        in_=xt_h[k * P:(k + 1) * P, tb * 4 * P:(tb + 1) * 4 * P])
                    xks.append(xk)
                vt4 = vpool.tile([P, 4 * ROW], f16, tag="vt4")
                for j in range(4):
                    ps = pspool.tile([P, ROW], f32, tag="ps")
                    for k in range(KT):
                        lhs = xks[k][:, j * P:(j + 1) * P]
                        nc.tensor.matmul(out=ps[:], lhsT=lhs, rhs=w_sb[k][:],
                                         start=(k == 0), stop=(k == KT - 1))
                    nc.vector.tensor_tensor(out=vt4[:, j * ROW:(j + 1) * ROW],
                                            in0=ps[:], in1=brow_sb[:],
                                            op=OP.add)
                nc.sync.dma_start(
                    out=table_h[tb * 4 * P:(tb + 1) * 4 * P, 0:ROW].rearrange(
                        "(j p) w -> p j w", p=P),
                    in_=vt4[:].rearrange("p (j w) -> p j w", w=ROW))

        for w in range(NW):
            dp = w * WROW + WROW - 1
            nc.sync.dma_start(out=table_h[dp:dp + 1, 0:ROW], in_=dm_sb[:])
        tc.strict_bb_all_engine_barrier()

        # ---------------- Phase B: per node-tile edge processing ---------
        with nc.named_scope("phaseB"), ExitStack() as bctx:
            gbytes = Cmax * EL * 2
            gbufs = 3 if gbytes * 3 <= 120 * 1024 else 2
            ipool = bctx.enter_context(tc.tile_pool(name="idx", bufs=3))
            gpool = bctx.enter_context(tc.tile_pool(name="g", bufs=gbufs))
            lpool = bctx.enter_context(tc.tile_pool(name="lr", bufs=3))
            epool = bctx.enter_context(tc.tile_pool(name="ee", bufs=3))
            spool = bctx.enter_context(tc.tile_pool(name="small", bufs=8))
            rpool = bctx.enter_context(tc.tile_pool(name="red", bufs=3))
            opool = bctx.enter_context(tc.tile_pool(name="on", bufs=4))
            fpool2 = bctx.enter_context(tc.tile_pool(name="fin", bufs=3))

            for r in range(R):
                Cr = int(C[r])
                E = Cr - 1                       # edge+pad columns
                if E == 0:
                    nc.sync.dma_start(out=out_h[r * P:(r + 1) * P, :],
                                      in_=fin0[:])
                    continue
                c0 = int(coff[r])
                idxt = ipool.tile([P, 8 * Cmax], i16, tag="idx")
                nc.sync.dma_start(out=idxt[:, 0:8 * Cr],
                                  in_=sidx_h[:, c0:c0 + 8 * Cr])
                G = gpool.tile([P, Cmax * EL], f16, tag="g")
                for (w, K, cb) in rounds[r]:
                    ncols = K + 1 if cb + K + 1 == Cr else K
                    # block includes self column iff it is the last block
                    wb = w * WROW
                    nc.gpsimd.dma_gather(
                        out_ap=G[:, cb * EL:(cb + ncols) * EL].rearrange(
                            "p (c e) -> p c e", e=EL),
                        in_ap=table_h[wb:wb + WROW, :],
                        idxs_ap=idxt[:, cb * 8:(cb + ncols) * 8],
                        num_idxs=ncols * P,
                        num_idxs_reg=ncols * P,
                        elem_size=EL,
                        single_packet=False,
                    )
                G3 = G[:, 0:Cr * EL].rearrange("p (c e) -> p c e", e=EL)
                f1c = G3[:, E:E + 1, D:D + 1]          # [128,1,1] self f1
                f2v = G3[:, 0:E, D + 1:D + 2]          # [128,E,1] edge f2
                lr = lpool.tile([P, Cmax], f32, tag="lr")
                nc.scalar.activation(out=lr[:, 0:E], in_=f2v, func=AF.Identity,
                                     bias=f1c, scale=1.0)
                lr2 = lpool.tile([P, Cmax], f32, tag="lr2")
                nc.vector.tensor_scalar(out=lr2[:, 0:E], in0=lr[:, 0:E],
                                        scalar1=LRELU_ALPHA, scalar2=None,
                                        op0=OP.mult)
                nc.vector.tensor_tensor(out=lr[:, 0:E], in0=lr[:, 0:E],
                                        in1=lr2[:, 0:E], op=OP.max)
                ee = epool.tile([P, Cmax], f16, tag="ee")
                ssum = spool.tile([P, 1], f32, tag="ssum")
                nc.scalar.activation(out=ee[:, 0:E], in_=lr[:, 0:E],
                                     func=AF.Exp, bias=esh_sb[:, 0:1],
                                     scale=1.0, accum_out=ssum[:])
                s2 = spool.tile([P, 1], f32, tag="s2")
                nc.vector.tensor_scalar(out=s2[:], in0=ssum[:], scalar1=1e-30,
                                        scalar2=None, op0=OP.add)
                rec = spool.tile([P, 1], f32, tag="rec")
                nc.vector.reciprocal(out=rec[:], in_=s2[:])

                # one broadcast multiply in place on the feature sub-rows
                gk = G3[:, 0:E, 0:D]
                eb = ee[:, 0:E].rearrange("p (s o) -> p s o", o=1) \
                    .to_broadcast([P, E, D])
                nc.vector.tensor_tensor(out=gk, in0=gk, in1=eb, op=OP.mult)

                # fold-halves tree over slots (fp16, in place on G)
                nsl = E
                while nsl > 2:
                    h = nsl // 2
                    lo = nsl - h
                    nc.vector.tensor_tensor(out=G3[:, 0:h, 0:D],
                                            in0=G3[:, 0:h, 0:D],
                                            in1=G3[:, lo:nsl, 0:D], op=OP.add)
                    nsl = lo
                red = rpool.tile([P, D], f32, tag="red")
                if nsl == 2:
                    nc.vector.tensor_tensor(out=red[:], in0=G3[:, 0:1, 0:D],
                                            in1=G3[:, 1:2, 0:D], op=OP.add)
                else:
                    nc.vector.tensor_copy(out=red[:], in_=G3[:, 0:1, 0:D])

                on = opool.tile([P, D], f16, tag="on")
                nc.vector.tensor_scalar(out=on[:], in0=red[:],
                                        scalar1=rec[:, 0:1], scalar2=None,
                                        op0=OP.mult)
                ob = opool.tile([P, D], f16, tag="ob")
                nc.vector.tensor_tensor(out=ob[:], in0=on[:], in1=bias_sb[:],
                                        op=OP.add)

                # fused elu(elu(x)) = max(x, exp(exp(min(x,0)) - 1) - 1)
                mm = opool.tile([P, D], f16, tag="mm")
                nc.vector.tensor_scalar(out=mm[:], in0=ob[:], scalar1=0.0,
                                        scalar2=None, op0=OP.min)
                ex = opool.tile([P, D], f16, tag="ex")
                nc.scalar.activation(out=ex[:], in_=mm[:], func=AF.Exp,
                                     bias=zero_sb[:, 0:1], scale=1.0)
                ex2 = opool.tile([P, D], f16, tag="ex2")
                nc.scalar.activation(out=ex2[:], in_=ex[:], func=AF.Exp,
                                     bias=negone_sb[:, 0:1], scale=1.0)
                e1 = opool.tile([P, D], f16, tag="e1")
                nc.vector.tensor_scalar(out=e1[:], in0=ex2[:], scalar1=-1.0,
                                        scalar2=None, op0=OP.add)
                fin = fpool2.tile([P, D], f32, tag="fin")
                nc.vector.tensor_tensor(out=fin[:], in0=ob[:], in1=e1[:],
                                        op=OP.max)
                nc.sync.dma_start(out=out_h[r * P:(r + 1) * P, :], in_=fin[:])

    nc.compile()
    return nc


def _run_kernel(X, edge_index, W, a1, b1, a2, b2, bias,
                n=N_NODES, in_dim=IN_DIM, trace=False):
    from concourse.bass_utils import run_bass_kernel_spmd

    dst = np.asarray(edge_index[0], dtype=np.int64)
    src = np.asarray(edge_index[1], dtype=np.int64)
    # divisible by 4096 => windows of npad/4 rows are 1024-aligned and
    # (for n=100000 -> npad=102400) fit the int16 gather index range
    npad = _ceil_to(n, NCORES * P * 4 * NW // 4)
    assert npad // NW <= 32768
    order, rounds, C, coff, TOTC, idx16 = _preprocess(dst, src, n, npad)

    exp_shift = 4.0 + max(0.0, float(b1) + float(b2))

    Xp = np.zeros((npad, in_dim), dtype=np.float32)
    Xp[:n] = X
    xt16 = np.ascontiguousarray(Xp[order].T.astype(np.float16))
    w1 = W.astype(np.float64) @ a1.astype(np.float64)
    w2 = W.astype(np.float64) @ a2.astype(np.float64)
    waug = np.concatenate(
        [W.astype(np.float32), w1[:, None].astype(np.float32),
         w2[:, None].astype(np.float32)], axis=1)
    waug16 = np.ascontiguousarray(waug.astype(np.float16))
    brow = np.zeros((1, ROW), dtype=np.float32)
    brow[0, OUT_DIM] = b1
    brow[0, OUT_DIM + 1] = b2
    bias1 = np.ascontiguousarray(bias.astype(np.float32).reshape(1, OUT_DIM))
    dummy = np.zeros((1, ROW), dtype=np.float16)
    dummy[0, OUT_DIM + 1] = DUMMY_F2

    nc = _build_program(npad, rounds, C, coff, TOTC, in_dim, exp_shift)

    in_maps = []
    for c in range(NCORES):
        in_maps.append({
            "xt": xt16, "waug": waug16, "brow": brow, "bias1": bias1,
            "dumrow": dummy, "sidx": np.ascontiguousarray(idx16[c]),
        })
    res = run_bass_kernel_spmd(nc, in_maps, list(range(NCORES)), trace=trace)
    _last_results["exec_time_ns"] = res.exec_time_ns
    _last_results["mean_exec_time_ns"] = res.mean_exec_time_ns
    _last_results["per_core_scope_times"] = res.per_core_scope_times

    R = len(rounds)
    out_full = np.empty((npad, OUT_DIM), dtype=np.float32)
    rr = np.repeat(np.arange(R), P)
    pp = np.tile(np.arange(P), R)
    for c in range(NCORES):
        pos = (rr * NCORES + c) * P + pp
        out_full[pos] = res.results[c]["out"]
    final = np.empty((npad, OUT_DIM), dtype=np.float32)
    final[order] = out_full
    return np.ascontiguousarray(final[:n])


def kernel(X, edge_index, W, a1, b1, a2, b2, bias):
    trace = bool(int(os.environ.get("GAT_KERNEL_TRACE", "0")))
    return _run_kernel(np.asarray(X, np.float32), np.asarray(edge_index),
                       np.asarray(W, np.float32),
                       np.asarray(a1, np.float32), np.float32(b1),
                       np.asarray(a2, np.float32), np.float32(b2),
                       np.asarray(bias, np.float32), trace=trace)


# revision 11
# speedup vs baseline: 1.7032x; 1.7032x over previous
"""GAT layer kernel for Trainium2 (8 NeuronCores, Bass/Tile).

Strategy:
  - Nodes are permuted by in-degree (host-side index preprocessing) so that
    128-node tiles have homogeneous degree; tiles are dealt round-robin to the
    8 cores so per-core edge counts balance and all cores share one SPMD
    instruction stream (per-tile padded degree K_r identical across cores).
  - Phase A (on device): table[n] = [seq_fts(n) (128, fp16) | f1(n)+b1 |
    f2(n)+b2] built with one PE matmul chain per 128-node tile against the
    host-side augmented weight matrix W_aug = [W | W@a1 | W@a2]; bias adds
    fused into the psum->fp16 copy; table written to HBM 4 tiles per DMA.
  - Phase B (on device): per node-tile, one indirect DMA per padded edge slot
    (plus a self slot providing f1) gathers table rows into a dense
    [node-partition x slot] SBUF layout.  SWDGE descriptor emission on the
    gpsimd Q7 (~8ns/row) is the hard bottleneck, so everything else hides
    under it.  Softmax over slots is free-dim work: ACT lrelu pieces -> ACT
    Exp with accum_out (denominator for free) -> one broadcast tensor_tensor
    multiply in place on the gathered tile -> fold-halves fp16 add tree ->
    normalize-after-aggregation -> fused elu(elu(x)).
    Dummy slots point at a table row with f2 = -60000 so exp() == 0 exactly.
"""

import os
import numpy as np

# ---- problem constants (self-contained; must match reference.py) ----
N_NODES = 100000
N_EDGES = 1600000
IN_DIM = 256
OUT_DIM = 128
LRELU_ALPHA = 0.2

NCORES = 8
P = 128
ROW = OUT_DIM + 2  # 128 feats + f1 + f2
DUMMY_F2 = -60000.0

_last_results = {}


def _ceil_to(x, m):
    return (x + m - 1) // m * m


def _preprocess(dst, src, n, npad):
    """Pure index preprocessing: degree-sort permutation, per-round padded
    degree K_r (exact max), and per-core gather index arrays."""
    ntiles = npad // P
    R = ntiles // NCORES

    deg = np.bincount(dst, minlength=npad).astype(np.int64)
    order = np.argsort(-deg, kind="stable")          # permuted pos -> node
    invpos = np.empty(npad, dtype=np.int64)
    invpos[order] = np.arange(npad)                  # node -> permuted pos

    posdeg = deg[order]                              # descending
    Kr = posdeg[np.arange(R) * (NCORES * P)].astype(np.int64)
    Sr = np.where(Kr > 0, Kr + 1, 0)                 # + self slot if nonempty
    offs = np.zeros(R + 1, dtype=np.int64)
    np.cumsum(P * Sr, out=offs[1:])
    TOT = int(offs[-1])

    # slot index of each edge within its destination node
    pos_d = invpos[dst]
    ordE = np.argsort(pos_d, kind="stable")
    pd_s = pos_d[ordE]
    sp_s = invpos[src][ordE]
    _, first, counts = np.unique(pd_s, return_index=True, return_counts=True)
    slot = np.arange(len(pd_s), dtype=np.int64) - np.repeat(first, counts)

    g = pd_s >> 7
    p = pd_s & 127
    c = (g % NCORES).astype(np.int64)
    r = g // NCORES
    flat = offs[r] + p * Sr[r] + slot

    idx_all = np.full((NCORES, max(TOT, 1)), npad, dtype=np.int32)
    idx_all[c, flat] = sp_s.astype(np.int32)

    # self slots: idx[p, K_r] = own permuted position
    ne = Sr > 0
    rr = np.repeat(np.arange(R)[ne], P)
    pp = np.tile(np.arange(P), int(ne.sum()))
    self_flat = offs[rr] + pp * Sr[rr] + Kr[rr]
    for cc in range(NCORES):
        own_pos = (rr * NCORES + cc) * P + pp
        idx_all[cc, self_flat] = own_pos.astype(np.int32)

    return order, Kr.tolist(), offs, TOT, idx_all


def _build_program(npad, Kr, offs, TOT, in_dim, exp_shift):
    import concourse.bass as bass
    import concourse.tile as tile
    from concourse import bacc, mybir
    from contextlib import ExitStack

    f16 = mybir.dt.float16
    f32 = mybir.dt.float32
    i32 = mybir.dt.int32
    AF = mybir.ActivationFunctionType
    OP = mybir.AluOpType
    D = OUT_DIM
    KT = in_dim // P
    R = len(Kr)
    rows_per_core = R * P

    nc = bacc.Bacc("TRN2", target_bir_lowering=False, debug=False,
                   num_devices=NCORES)
    xt_h = nc.declare_dram_parameter("xt", [in_dim, npad], f16, isOutput=False)
    waug_h = nc.declare_dram_parameter("waug", [in_dim, ROW], f16,
                                       isOutput=False)
    brow_h = nc.declare_dram_parameter("brow", [1, ROW], f32, isOutput=False)
    bias_h = nc.declare_dram_parameter("bias1", [1, D], f32, isOutput=False)
    dummy_h = nc.declare_dram_parameter("dumrow", [1, ROW], f16, isOutput=False)
    sidx_h = nc.declare_dram_parameter("sidx", [max(TOT, 1)], i32,
                                       isOutput=False)
    out_h = nc.declare_dram_parameter("out", [rows_per_core, D], f32,
                                      isOutput=True)

    table_h = nc.dram_tensor("table", [npad + 1, ROW], f16)

    with tile.TileContext(nc) as tc, ExitStack() as ctx:
        cpool = ctx.enter_context(tc.tile_pool(name="consts", bufs=1))
        w_sb = [cpool.tile([P, ROW], f16, name=f"wsb{k}", tag=f"w{k}")
                for k in range(KT)]
        for k in range(KT):
            nc.sync.dma_start(out=w_sb[k][:], in_=waug_h[k * P:(k + 1) * P, :])
        brow_sb = cpool.tile([P, ROW], f32, tag="brow")
        nc.sync.dma_start(out=brow_sb[:],
                          in_=brow_h[0:1, :].to_broadcast([P, ROW]))
        bias_sb = cpool.tile([P, D], f32, tag="bias128")
        nc.sync.dma_start(out=bias_sb[:],
                          in_=bias_h[0:1, :].to_broadcast([P, D]))
        esh_sb = cpool.tile([P, 1], f32, tag="eshcol")
        nc.vector.memset(esh_sb[:], -float(exp_shift))
        zero_sb = cpool.tile([P, 1], f32, tag="zerocol")
        nc.vector.memset(zero_sb[:], 0.0)
        negone_sb = cpool.tile([P, 1], f32, tag="negonecol")
        nc.vector.memset(negone_sb[:], -1.0)
        dm_sb = cpool.tile([1, ROW], f16, tag="dummy")
        nc.sync.dma_start(out=dm_sb[:], in_=dummy_h[:, :])

        # fin0 = elu(elu(bias)) for empty rounds
        ob0 = cpool.tile([P, D], f16, tag="ob0")
        nc.vector.tensor_copy(out=ob0[:], in_=bias_sb[:])
        mm0 = cpool.tile([P, D], f16, tag="mm0")
        nc.vector.tensor_scalar(out=mm0[:], in0=ob0[:], scalar1=0.0,
                                scalar2=None, op0=OP.min)
        ex0 = cpool.tile([P, D], f16, tag="ex0")
        nc.scalar.activation(out=ex0[:], in_=mm0[:], func=AF.Exp,
                             bias=zero_sb[:, 0:1], scale=1.0)
        ex20 = cpool.tile([P, D], f16, tag="ex20")
        nc.scalar.activation(out=ex20[:], in_=ex0[:], func=AF.Exp,
                             bias=negone_sb[:, 0:1], scale=1.0)
        e10 = cpool.tile([P, D], f16, tag="e10")
        nc.vector.tensor_scalar(out=e10[:], in0=ex20[:], scalar1=-1.0,
                                scalar2=None, op0=OP.add)
        fin0 = cpool.tile([P, D], f32, tag="fin0")
        nc.vector.tensor_tensor(out=fin0[:], in0=ob0[:], in1=e10[:], op=OP.max)

        # ---------------- Phase A: build the table (node-major) ----------
        with nc.named_scope("phaseA"), ExitStack() as actx:
            xpool = actx.enter_context(tc.tile_pool(name="x", bufs=6))
            pspool = actx.enter_context(
                tc.tile_pool(name="psA", bufs=6, space="PSUM"))
            vpool = actx.enter_context(tc.tile_pool(name="vtile", bufs=4))

            ntile_all = npad // P
            for tb in range(ntile_all // 4):
                xks = []
                for k in range(KT):
                    xk = xpool.tile([P, 4 * P], f16, tag="xk", name=f"xk{k}")
                    nc.scalar.dma_start(
                        out=xk[:],
                        in_=xt_h[k * P:(k + 1) * P, tb * 4 * P:(tb + 1) * 4 * P])
                    xks.append(xk)
                vt4 = vpool.tile([P, 4 * ROW], f16, tag="vt4")
                for j in range(4):
                    ps = pspool.tile([P, ROW], f32, tag="ps")
                    for k in range(KT):
                        lhs = xks[k][:, j * P:(j + 1) * P]
                        nc.tensor.matmul(out=ps[:], lhsT=lhs, rhs=w_sb[k][:],
                                         start=(k == 0), stop=(k == KT - 1))
                    nc.vector.tensor_tensor(out=vt4[:, j * ROW:(j + 1) * ROW],
                                            in0=ps[:], in1=brow_sb[:],
                                            op=OP.add)
                nc.sync.dma_start(
                    out=table_h[tb * 4 * P:(tb + 1) * 4 * P, :].rearrange(
                        "(j p) w -> p j w", p=P),
                    in_=vt4[:].rearrange("p (j w) -> p j w", w=ROW))

        nc.sync.dma_start(out=table_h[npad:npad + 1, :], in_=dm_sb[:])
        tc.strict_bb_all_engine_barrier()

        # ---------------- Phase B: per node-tile edge processing ---------
        with nc.named_scope("phaseB"), ExitStack() as bctx:
            Kmax = max(Kr)
            Smax = Kmax + 1
            gbytes = Smax * ROW * 2
            gbufs = 3 if gbytes * 3 <= 110 * 1024 else 2
            ipool = bctx.enter_context(tc.tile_pool(name="idx", bufs=3))
            gpool = bctx.enter_context(tc.tile_pool(name="g", bufs=gbufs))
            lpool = bctx.enter_context(tc.tile_pool(name="lr", bufs=3))
            epool = bctx.enter_context(tc.tile_pool(name="ee", bufs=3))
            spool = bctx.enter_context(tc.tile_pool(name="small", bufs=8))
            rpool = bctx.enter_context(tc.tile_pool(name="red", bufs=3))
            opool = bctx.enter_context(tc.tile_pool(name="on", bufs=4))
            fpool2 = bctx.enter_context(tc.tile_pool(name="fin", bufs=3))

            for r in range(R):
                K = Kr[r]
                if K == 0:
                    nc.sync.dma_start(out=out_h[r * P:(r + 1) * P, :],
                                      in_=fin0[:])
                    continue
                S = K + 1
                off = int(offs[r])
                idxt = ipool.tile([P, Smax], i32, tag="idx")
                nc.sync.dma_start(
                    out=idxt[:, 0:S],
                    in_=sidx_h[off:off + P * S].rearrange("(p s) -> p s", s=S))
                G = gpool.tile([P, Smax * ROW], f16, tag="g")
                for k in range(S):
                    nc.gpsimd.indirect_dma_start(
                        out=G[:, k * ROW:(k + 1) * ROW],
                        out_offset=None,
                        in_=table_h[:, :],
                        in_offset=bass.IndirectOffsetOnAxis(
                            ap=idxt[:, k:k + 1], axis=0),
                    )
                G3 = G[:, 0:S * ROW].rearrange("p (s w) -> p s w", w=ROW)
                f1c = G3[:, K:K + 1, D:D + 1]          # [128,1,1] self f1
                f2v = G3[:, 0:K, D + 1:D + 2]          # [128,K,1] edge f2
                lr = lpool.tile([P, Kmax], f32, tag="lr")
                nc.scalar.activation(out=lr[:, 0:K], in_=f2v, func=AF.Identity,
                                     bias=f1c, scale=1.0)
                lr2 = lpool.tile([P, Kmax], f32, tag="lr2")
                nc.vector.tensor_scalar(out=lr2[:, 0:K], in0=lr[:, 0:K],
                                        scalar1=LRELU_ALPHA, scalar2=None,
                                        op0=OP.mult)
                nc.vector.tensor_tensor(out=lr[:, 0:K], in0=lr[:, 0:K],
                                        in1=lr2[:, 0:K], op=OP.max)
                ee = epool.tile([P, Kmax], f16, tag="ee")
                ssum = spool.tile([P, 1], f32, tag="ssum")
                nc.scalar.activation(out=ee[:, 0:K], in_=lr[:, 0:K],
                                     func=AF.Exp, bias=esh_sb[:, 0:1],
                                     scale=1.0, accum_out=ssum[:])
                s2 = spool.tile([P, 1], f32, tag="s2")
                nc.vector.tensor_scalar(out=s2[:], in0=ssum[:], scalar1=1e-30,
                                        scalar2=None, op0=OP.add)
                rec = spool.tile([P, 1], f32, tag="rec")
                nc.vector.reciprocal(out=rec[:], in_=s2[:])

                # one broadcast multiply in place on the feature sub-rows
                gk = G3[:, 0:K, 0:D]
                eb = ee[:, 0:K].rearrange("p (s o) -> p s o", o=1) \
                    .to_broadcast([P, K, D])
                nc.vector.tensor_tensor(out=gk, in0=gk, in1=eb, op=OP.mult)

                # fold-halves tree over slots (fp16, in place on G)
                nsl = K
                while nsl > 2:
                    h = nsl // 2
                    lo = nsl - h
                    nc.vector.tensor_tensor(out=G3[:, 0:h, 0:D],
                                            in0=G3[:, 0:h, 0:D],
                                            in1=G3[:, lo:nsl, 0:D], op=OP.add)
                    nsl = lo
                red = rpool.tile([P, D], f32, tag="red")
                if nsl == 2:
                    nc.vector.tensor_tensor(out=red[:], in0=G3[:, 0:1, 0:D],
                                            in1=G3[:, 1:2, 0:D], op=OP.add)
                else:
                    nc.vector.tensor_copy(out=red[:], in_=G3[:, 0:1, 0:D])

                on = opool.tile([P, D], f16, tag="on")
                nc.vector.tensor_scalar(out=on[:], in0=red[:],
                                        scalar1=rec[:, 0:1], scalar2=None,
                                        op0=OP.mult)
                ob = opool.tile([P, D], f16, tag="ob")
                nc.vector.tensor_tensor(out=ob[:], in0=on[:], in1=bias_sb[:],
                                        op=OP.add)

                # fused elu(elu(x)) = max(x, exp(exp(min(x,0)) - 1) - 1)
                mm = opool.tile([P, D], f16, tag="mm")
                nc.vector.tensor_scalar(out=mm[:], in0=ob[:], scalar1=0.0,
                                        scalar2=None, op0=OP.min)
                ex = opool.tile([P, D], f16, tag="ex")
                nc.scalar.activation(out=ex[:], in_=mm[:], func=AF.Exp,
                                     bias=zero_sb[:, 0:1], scale=1.0)
                ex2 = opool.tile([P, D], f16, tag="ex2")
                nc.scalar.activation(out=ex2[:], in_=ex[:], func=AF.Exp,
                                     bias=negone_sb[:, 0:1], scale=1.0)
                e1 = opool.tile([P, D], f16, tag="e1")
                nc.vector.tensor_scalar(out=e1[:], in0=ex2[:], scalar1=-1.0,
                                        scalar2=None, op0=OP.add)
                fin = fpool2.tile([P, D], f32, tag="fin")
                nc.vector.tensor_tensor(out=fin[:], in0=ob[:], in1=e1[:],
                                        op=OP.max)
                nc.sync.dma_start(out=out_h[r * P:(r + 1) * P, :], in_=fin[:])

    nc.compile()
    return nc


def _run_kernel(X, edge_index, W, a1, b1, a2, b2, bias,
                n=N_NODES, in_dim=IN_DIM, trace=False):
    from concourse.bass_utils import run_bass_kernel_spmd

    dst = np.asarray(edge_index[0], dtype=np.int64)
    src = np.asarray(edge_index[1], dtype=np.int64)
    npad = _ceil_to(n, NCORES * P * 4)  # divisible by 1024 and 512
    order, Kr, offs, TOT, idx_all = _preprocess(dst, src, n, npad)

    exp_shift = 4.0 + max(0.0, float(b1) + float(b2))

    Xp = np.zeros((npad, in_dim), dtype=np.float32)
    Xp[:n] = X
    xt16 = np.ascontiguousarray(Xp[order].T.astype(np.float16))
    w1 = W.astype(np.float64) @ a1.astype(np.float64)
    w2 = W.astype(np.float64) @ a2.astype(np.float64)
    waug = np.concatenate(
        [W.astype(np.float32), w1[:, None].astype(np.float32),
         w2[:, None].astype(np.float32)], axis=1)
    waug16 = np.ascontiguousarray(waug.astype(np.float16))
    brow = np.zeros((1, ROW), dtype=np.float32)
    brow[0, OUT_DIM] = b1
    brow[0, OUT_DIM + 1] = b2
    bias1 = np.ascontiguousarray(bias.astype(np.float32).reshape(1, OUT_DIM))
    dummy = np.zeros((1, ROW), dtype=np.float16)
    dummy[0, OUT_DIM + 1] = DUMMY_F2

    nc = _build_program(npad, Kr, offs, TOT, in_dim, exp_shift)

    in_maps = []
    for c in range(NCORES):
        in_maps.append({
            "xt": xt16, "waug": waug16, "brow": brow, "bias1": bias1,
            "dumrow": dummy, "sidx": np.ascontiguousarray(idx_all[c]),
        })
    res = run_bass_kernel_spmd(nc, in_maps, list(range(NCORES)), trace=trace)
    _last_results["exec_time_ns"] = res.exec_time_ns
    _last_results["mean_exec_time_ns"] = res.mean_exec_time_ns
    _last_results["per_core_scope_times"] = res.per_core_scope_times

    R = len(Kr)
    out_full = np.empty((npad, OUT_DIM), dtype=np.float32)
    rr = np.repeat(np.arange(R), P)
    pp = np.tile(np.arange(P), R)
    for c in range(NCORES):
        pos = (rr * NCORES + c) * P + pp
        out_full[pos] = res.results[c]["out"]
    final = np.empty((npad, OUT_DIM), dtype=np.float32)
    final[order] = out_full
    return np.ascontiguousarray(final[:n])


def kernel(X, edge_index, W, a1, b1, a2, b2, bias):
    trace = bool(int(os.environ.get("GAT_KERNEL_TRACE", "0")))
    return _run_kernel(np.asarray(X, np.float32), np.asarray(edge_index),
                       np.asarray(W, np.float32),
                       np.asarray(a1, np.float32), np.float32(b1),
                       np.asarray(a2, np.float32), np.float32(b2),
                       np.asarray(bias, np.float32), trace=trace)
